# revision 1
# baseline (speedup 1.0000x reference)
"""JukeboxAttention Trainium2 kernel.

Shards the B*BLOCKS=32 independent attention blocks across 8 NeuronCores
(4 blocks = 2048 tokens per core); weights are replicated. Each core runs:
  qkv^T = W^T x^T (fp32r matmuls), per-head causal block attention in
  transposed [k, q] layout, then out = ctx @ c_proj_w + b.
"""

import sys

sys.path.insert(0, "/opt/trn_rl_repo")

import numpy as np

B, L, E = 2, 8192, 2048
HEADS, HD = 16, 128
BLOCKS, BC = 16, 512
SCALE2 = float(HD) ** -0.5  # (hd^-0.25)^2 applied to q side
NCORES = 8
BPC = B * BLOCKS // NCORES  # blocks per core = 4
T = BPC * BC  # tokens per core = 2048
ET = E // 128  # 16 contraction tiles


def _build_nc(reps=1):
    import concourse.bass as bass  # noqa: F401
    from concourse import bacc, mybir, tile

    f32 = mybir.dt.float32
    R = mybir.dt.float32r
    Act = mybir.ActivationFunctionType

    nc = bacc.Bacc("TRN2", target_bir_lowering=False, debug=False)

    xs = nc.dram_tensor("xs", [T, E], f32, kind="ExternalInput").ap()
    waq = nc.dram_tensor("waq", [E, 3 * E], f32, kind="ExternalInput").ap()
    cab = nc.dram_tensor("cab", [128, 3 * ET], f32, kind="ExternalInput").ap()
    wp = nc.dram_tensor("wp", [E, E], f32, kind="ExternalInput").ap()
    cpb = nc.dram_tensor("cpb", [E], f32, kind="ExternalInput").ap()
    maskt = nc.dram_tensor("maskt", [128, 4 * BC], f32, kind="ExternalInput").ap()
    ident = nc.dram_tensor("ident", [128, 128], f32, kind="ExternalInput").ap()
    out = nc.dram_tensor("out", [T, E], f32, kind="ExternalOutput").ap()

    with tile.TileContext(nc) as tc:
        with (
            tc.tile_pool(name="const", bufs=1) as const,
            tc.tile_pool(name="xpool", bufs=2) as xpool,
            tc.tile_pool(name="wload", bufs=3) as wload,
            tc.tile_pool(name="xt", bufs=1) as xtp,
            tc.tile_pool(name="ctxt", bufs=1) as ctxp,
            tc.tile_pool(name="qkv", bufs=6) as qkvp,
            tc.tile_pool(name="vhead", bufs=2) as vhp,
            tc.tile_pool(name="probs", bufs=6) as prp,
            tc.tile_pool(name="sums", bufs=2) as sup,
            tc.tile_pool(name="sumh", bufs=3) as suh,
            tc.tile_pool(name="rsb", bufs=3) as rsp,
            tc.tile_pool(name="rbc", bufs=2) as rbp,
            tc.tile_pool(name="outp", bufs=3) as outp,
            tc.tile_pool(name="psb", bufs=6, space="PSUM") as psb,
            tc.tile_pool(name="psr", bufs=2, space="PSUM") as psr,
        ):
            # ---- constants ----
            ident_sb = const.tile([128, 128], f32, tag="ident")
            nc.sync.dma_start(out=ident_sb, in_=ident)
            mask_sb = const.tile([128, 4 * BC], f32, tag="mask")
            nc.sync.dma_start(out=mask_sb, in_=maskt)
            cab_sb = const.tile([128, 3 * ET], f32, tag="cab")
            nc.sync.dma_start(out=cab_sb, in_=cab)
            pbias_bc = const.tile([128, E], f32, tag="pbias")
            pb_ap = bass.AP(tensor=cpb.tensor, offset=cpb.offset,
                            ap=[[0, 128], [1, E]])
            nc.gpsimd.dma_start(out=pbias_bc, in_=pb_ap)
            ones_col_f = const.tile([128, 1], f32, tag="ones_col")
            nc.vector.memset(ones_col_f, 1.0)
            ones_row_f = const.tile([1, 128], f32, tag="ones_row")
            nc.vector.memset(ones_row_f, 1.0)
            ones_col = ones_col_f.bitcast(R)
            ones_row = ones_row_f.bitcast(R)

            for blk_i in range(BPC * reps):
                blk = blk_i % BPC
                t0 = blk * BC
                # ---- phase A: Xt = x^T (per 128x128 PE transpose) ----
                Xt = xtp.tile([128, ET, BC], R, tag="xt")
                for m in range(4):
                    x_t = xpool.tile([128, E], f32, tag="x")
                    nc.sync.dma_start(out=x_t, in_=xs[t0 + m * 128: t0 + (m + 1) * 128, :])
                    for g in range(ET // 4):
                        ps4 = psb.tile([128, 4, 128], f32, tag="ps", name=f"ps4_{g}")
                        for j in range(4):
                            et = g * 4 + j
                            nc.tensor.transpose(ps4[:, j, :],
                                                x_t[:, et * 128:(et + 1) * 128], ident_sb)
                        dst = Xt[:, g * 4:(g + 1) * 4, m * 128:(m + 1) * 128]
                        if (m + g) % 2 == 0:
                            nc.vector.tensor_copy(dst, ps4)
                        else:
                            nc.scalar.copy(dst, ps4)

                # ---- phase B: per-head qkv + attention ----
                ctxT = ctxp.tile([128, HEADS, BC], R, tag="ctxt")
                sums_buf = sup.tile([HEADS, BC], f32, tag="sums")
                for h in range(HEADS):
                    sb3 = []
                    for ft, scale in ((h, SCALE2), (ET + h, 1.0), (2 * ET + h, 1.0)):
                        wst = wload.tile([128, ET, 128], R, tag="w")
                        nc.sync.dma_start(
                            out=wst,
                            in_=waq[:, ft * 128:(ft + 1) * 128]
                            .rearrange("(et p) j -> p et j", p=128).bitcast(R),
                        )
                        ps = psb.tile([128, BC], f32, tag="ps")
                        for et in range(ET):
                            nc.tensor.matmul(ps, lhsT=wst[:, et, :], rhs=Xt[:, et, :],
                                             start=(et == 0), stop=(et == ET - 1))
                        sb = qkvp.tile([128, BC], R, tag="qkv")
                        nc.scalar.activation(sb, ps, Act.Identity,
                                             bias=cab_sb[:, ft:ft + 1], scale=scale)
                        sb3.append(sb)
                    q_sb, k_sb, v_sb = sb3

                    # v in natural [token, hd] layout via PE transpose
                    v_head = vhp.tile([128, 4, 128], R, tag="vh")
                    for kt in range(4):
                        pt = psb.tile([128, 128], f32, tag="ps")
                        nc.tensor.transpose(pt, v_sb.bitcast(f32)[:, kt * 128:(kt + 1) * 128],
                                            ident_sb)
                        nc.vector.tensor_copy(v_head[:, kt, :], pt)

                    # scores^T [k, q] -> exp -> mask
                    pbs = []
                    for kt in range(4):
                        ps_s = psb.tile([128, BC], f32, tag="ps")
                        nc.tensor.matmul(ps_s, lhsT=k_sb[:, kt * 128:(kt + 1) * 128],
                                         rhs=q_sb, start=True, stop=True)
                        pb = prp.tile([128, BC], R, tag="pb")
                        nc.scalar.activation(pb, ps_s, Act.Exp)
                        nc.vector.tensor_mul(pb, pb, mask_sb[:, kt * BC:(kt + 1) * BC])
                        pbs.append(pb)

                    # denominators: ones^T @ probsT
                    ps_sum = psr.tile([1, BC], f32, tag="psr")
                    for kt in range(4):
                        nc.tensor.matmul(ps_sum, lhsT=ones_col, rhs=pbs[kt],
                                         start=(kt == 0), stop=(kt == 3))
                    sums_h = suh.tile([1, BC], f32, tag="sumh")
                    nc.scalar.copy(sums_h, ps_sum)
                    nc.sync.dma_start(out=sums_buf[h:h + 1, :], in_=sums_h)

                    # ctx^T accumulate
                    ps_c = psb.tile([128, BC], f32, tag="ps")
                    for kt in range(4):
                        nc.tensor.matmul(ps_c, lhsT=v_head[:, kt, :], rhs=pbs[kt],
                                         start=(kt == 0), stop=(kt == 3))
                    nc.scalar.copy(ctxT[:, h, :], ps_c)

                # ---- phase C: normalize ctx^T ----
                recip = sup.tile([HEADS, BC], f32, tag="recip")
                nc.vector.reciprocal(recip, sums_buf)
                for h in range(HEADS):
                    rsb = rsp.tile([1, BC], R, tag="rsb")
                    nc.sync.dma_start(out=rsb, in_=recip[h:h + 1, :].bitcast(R))
                    ps_b = psr.tile([128, BC], f32, tag="psr")
                    nc.tensor.matmul(ps_b, lhsT=ones_row, rhs=rsb, start=True, stop=True)
                    rbc = rbp.tile([128, BC], f32, tag="rbc")
                    nc.scalar.copy(rbc, ps_b)
                    nc.vector.tensor_mul(ctxT[:, h, :], ctxT[:, h, :], rbc)

                # ---- phase D: out = ctx @ c_proj_w + b ----
                for f in range(4):
                    ps_os = [psb.tile([128, BC], f32, tag="ps", name=f"ps_o{m}")
                             for m in range(4)]
                    for dg in range(4):
                        wpg = wload.tile([128, 4, BC], R, tag="w")
                        nc.sync.dma_start(
                            out=wpg,
                            in_=wp[dg * 512:(dg + 1) * 512, f * 512:(f + 1) * 512]
                            .rearrange("(dt p) j -> p dt j", p=128).bitcast(R),
                        )
                        for dtl in range(4):
                            dt = dg * 4 + dtl
                            for m in range(4):
                                nc.tensor.matmul(
                                    ps_os[m],
                                    lhsT=ctxT[:, dt, m * 128:(m + 1) * 128],
                                    rhs=wpg[:, dtl, :],
                                    start=(dt == 0), stop=(dt == 15),
                                )
                    for m in range(4):
                        osb = outp.tile([128, BC], f32, tag="osb")
                        nc.vector.tensor_add(osb, ps_os[m], pbias_bc[:, f * 512:(f + 1) * 512])
                        nc.sync.dma_start(
                            out=out[t0 + m * 128: t0 + (m + 1) * 128, f * 512:(f + 1) * 512],
                            in_=osb,
                        )
    nc.compile()
    return nc


_NC = {}


def _get_nc(reps=1):
    if reps not in _NC:
        _NC[reps] = _build_nc(reps)
    return _NC[reps]


def make_in_maps(x, c_attn_w, c_attn_b, c_proj_w, c_proj_b):
    x = np.asarray(x, np.float32)
    c_attn_w = np.ascontiguousarray(c_attn_w, np.float32)
    c_proj_w = np.ascontiguousarray(c_proj_w, np.float32)
    b_mod = np.asarray(c_attn_b, np.float32).copy()
    b_mod[:E] *= SCALE2
    cab = np.ascontiguousarray(b_mod.reshape(3 * ET, 128).T)

    # mask[p, kt*BC + c] = 1 if query c >= key kt*128+p else 0
    p = np.arange(128)[:, None]
    c = np.arange(BC)[None, :]
    maskt = np.concatenate(
        [(c >= kt * 128 + p).astype(np.float32) for kt in range(4)], axis=1)
    maskt = np.ascontiguousarray(maskt)
    ident = np.eye(128, dtype=np.float32)

    xr = x.reshape(B * BLOCKS, BC, E)
    in_maps = []
    for core in range(NCORES):
        xs = np.ascontiguousarray(xr[core * BPC:(core + 1) * BPC].reshape(T, E))
        in_maps.append({
            "xs": xs, "waq": c_attn_w, "cab": cab, "wp": c_proj_w,
            "cpb": np.ascontiguousarray(c_proj_b, np.float32),
            "maskt": maskt, "ident": ident,
        })
    return in_maps


def kernel(x, c_attn_w, c_attn_b, c_proj_w, c_proj_b):
    from concourse import bass_utils

    nc = _get_nc()
    in_maps = make_in_maps(x, c_attn_w, c_attn_b, c_proj_w, c_proj_b)
    res = bass_utils.run_bass_kernel_spmd(nc, in_maps, core_ids=list(range(NCORES)))
    outs = [res.results[c]["out"] for c in range(NCORES)]
    full = np.concatenate(outs, axis=0).reshape(B, L, E).astype(np.float32)
    return full

